# revision 4
# baseline (speedup 1.0000x reference)
"""DF11-compressed linear layer on 8 Trainium2 NeuronCores.

y = x @ W^T + bias, where W [4096, 4096] bf16 is decoded on-device from
DF11 compression: per-element exponent code (exp_idx -> lut_exp) plus
packed sign+mantissa byte.

Sharding (column-parallel): out_features split 8 ways; each core decodes
its [512, 4096] compressed shard to bf16 and matmuls against the shared
activations. Outputs are concatenated on the host.

Per-core pipeline:
  1. gpsimd DMA loads the int32 compressed shard, casting to uint16.
  2. DVE rebuilds bf16 bits with 4 integer ops per element:
       sel  = (v >= 128) * 0x7F80          # sign bit relocation
       bexp = k * 128 + (lut_base << 7)    # biased exponent field
       bits = (v + sel) + bexp             # sign|exp|mantissa
     then bitcasts the uint16 tile to bf16 (W stored [o, i]).
  3. TensorE transposes 128x128 blocks (bit-exact) into PSUM; ScalarE
     copies them to SBUF giving W^T [i, o] tiles.
  4. TensorE GEMM: y[b, o] accumulates over 32 k-tiles with x^T as the
     stationary operand.
  5. DVE adds the broadcast bias; result DMAs out as y [16, 512] f32.
"""

import numpy as np
import ml_dtypes

import concourse.mybir as mybir
import concourse.tile as tile
from concourse import bacc
from concourse.bass_utils import run_bass_kernel_spmd
from concourse.masks import make_identity

O = 4096          # out_features
I = 4096          # in_features
B = 16            # batch
N_CORES = 8
OS = O // N_CORES  # 512 out_features per core
P = 128
N_OT = OS // P     # o-tiles per core (4)
N_KT = I // P      # k-tiles (32)


def _build_program():
    nc = bacc.Bacc("TRN2", target_bir_lowering=False)

    ei_d = nc.dram_tensor("ei", [OS, I], mybir.dt.int32, kind="ExternalInput")
    sm_d = nc.dram_tensor("sm", [OS, I], mybir.dt.int32, kind="ExternalInput")
    xT_d = nc.dram_tensor("xT", [I, B], mybir.dt.bfloat16, kind="ExternalInput")
    bias_d = nc.dram_tensor("bias", [OS], mybir.dt.float32, kind="ExternalInput")
    # (lut_exp[0] << 7) replicated per partition so the exponent offset is a
    # runtime value, keeping one compiled program for any lut base.
    basec_d = nc.dram_tensor("basec", [P, 1], mybir.dt.float32, kind="ExternalInput")
    y_d = nc.dram_tensor("y", [B, OS], mybir.dt.float32, kind="ExternalOutput")

    ei_t = ei_d[:].rearrange("(t p) i -> t p i", p=P)
    sm_t = sm_d[:].rearrange("(t p) i -> t p i", p=P)

    with tile.TileContext(nc) as tc:
        with (
            tc.tile_pool(name="const", bufs=1) as cpool,
            tc.tile_pool(name="wt", bufs=1) as wtpool,
            tc.tile_pool(name="dec", bufs=2) as dec,
            tc.tile_pool(name="psum_t", bufs=2, space="PSUM") as pst,
            tc.tile_pool(name="psum_y", bufs=1, space="PSUM") as psy,
        ):
            ident = cpool.tile([P, P], mybir.dt.bfloat16)
            make_identity(nc, ident[:])
            basec = cpool.tile([P, 1], mybir.dt.float32)
            nc.sync.dma_start(basec[:], basec_d[:])
            xT_sb = cpool.tile([P, N_KT, B], mybir.dt.bfloat16)
            nc.sync.dma_start(xT_sb[:], xT_d[:].rearrange("(j p) b -> p j b", p=P))
            bias_bc = cpool.tile([B, OS], mybir.dt.float32)
            nc.sync.dma_start(bias_bc[:], bias_d[None, :].to_broadcast((B, OS)))

            # W^T staging: [i-partition, k-tile, o] bf16
            wt_sb = wtpool.tile([P, N_KT, OS], mybir.dt.bfloat16)
            y_ps = psy.tile([B, OS], mybir.dt.float32)

            for t in range(N_OT):
                v16 = dec.tile([P, I], mybir.dt.uint16, tag="v16")
                k16 = dec.tile([P, I], mybir.dt.uint16, tag="k16")
                nc.gpsimd.dma_start(v16[:], sm_t[t])   # int32 -> uint16 cast
                nc.gpsimd.dma_start(k16[:], ei_t[t])

                sel = dec.tile([P, I], mybir.dt.uint16, tag="sel")
                nc.vector.tensor_scalar(
                    out=sel[:], in0=v16[:], scalar1=128, scalar2=0x7F80,
                    op0=mybir.AluOpType.is_ge, op1=mybir.AluOpType.mult,
                )
                bexp = dec.tile([P, I], mybir.dt.uint16, tag="bexp")
                nc.vector.tensor_scalar(
                    out=bexp[:], in0=k16[:], scalar1=128, scalar2=basec[:, 0:1],
                    op0=mybir.AluOpType.mult, op1=mybir.AluOpType.add,
                )
                tmp = dec.tile([P, I], mybir.dt.uint16, tag="tmp")
                nc.vector.tensor_tensor(
                    out=tmp[:], in0=v16[:], in1=sel[:], op=mybir.AluOpType.add
                )
                bits = dec.tile([P, I], mybir.dt.uint16, tag="bits")
                nc.vector.tensor_tensor(
                    out=bits[:], in0=tmp[:], in1=bexp[:], op=mybir.AluOpType.add
                )
                W = bits[:].bitcast(mybir.dt.bfloat16)  # [P(o), I(i)]

                # transpose 128x128 blocks, 8 per PSUM bank, batch-copy out
                for g in range(N_KT // 8):
                    pt = pst.tile([P, 8, P], mybir.dt.bfloat16, tag="pt")
                    for jj in range(8):
                        j = g * 8 + jj
                        nc.tensor.transpose(
                            pt[:, jj, :], W[:, j * P:(j + 1) * P], ident[:]
                        )
                    nc.scalar.copy(
                        wt_sb[:, g * 8:(g + 1) * 8, t * P:(t + 1) * P], pt[:]
                    )

            for j in range(N_KT):
                nc.tensor.matmul(
                    y_ps[:], xT_sb[:, j, :], wt_sb[:, j, :],
                    start=(j == 0), stop=(j == N_KT - 1),
                )

            y_sb = cpool.tile([B, OS], mybir.dt.float32)
            nc.vector.tensor_tensor(
                out=y_sb[:], in0=y_ps[:], in1=bias_bc[:], op=mybir.AluOpType.add
            )
            nc.sync.dma_start(y_d[:], y_sb[:])

    nc.compile()
    return nc


_NC_CACHE = None


def _get_program():
    global _NC_CACHE
    if _NC_CACHE is None:
        _NC_CACHE = _build_program()
    return _NC_CACHE


def kernel(x, exp_idx, sign_mant, lut_exp, bias, trace=False, tmpdir=None):
    x = np.asarray(x, dtype=np.float32)
    exp_idx = np.ascontiguousarray(np.asarray(exp_idx, dtype=np.int32))
    sign_mant = np.ascontiguousarray(np.asarray(sign_mant, dtype=np.int32))
    lut_exp = np.asarray(lut_exp, dtype=np.int32)
    bias = np.ascontiguousarray(np.asarray(bias, dtype=np.float32))

    # The on-device decode computes exponent = code + base. When the LUT is
    # affine (it is arange-filled by construction) the codes pass through
    # unchanged; otherwise resolve the 32-entry LUT on the host so the device
    # math stays exact for arbitrary LUT contents.
    if np.array_equal(lut_exp, lut_exp[0] + np.arange(len(lut_exp), dtype=np.int32)):
        codes = exp_idx
        base = int(lut_exp[0])
    else:
        codes = np.ascontiguousarray(lut_exp[exp_idx].astype(np.int32))
        base = 0

    basec = np.full((P, 1), float(base << 7), dtype=np.float32)
    xT = np.ascontiguousarray(x.astype(ml_dtypes.bfloat16).T)

    in_maps = []
    for c in range(N_CORES):
        sl = slice(c * OS, (c + 1) * OS)
        in_maps.append({
            "ei": codes[sl],
            "sm": sign_mant[sl],
            "xT": xT,
            "bias": bias[sl],
            "basec": basec,
        })

    nc = _get_program()
    res = run_bass_kernel_spmd(
        nc, in_maps, core_ids=list(range(N_CORES)), trace=trace, tmpdir=tmpdir
    )
    y = np.concatenate([r["y"] for r in res.results], axis=1)
    if trace:
        kernel.last_results = res
    return y
